# revision 12
# baseline (speedup 1.0000x reference)
"""Trainium2 Bass kernel for nn_CrossAttnBlock (sparse banded cross-attention).

Reference computation (B=4, T=4096, K=256, D=1024, H=16, hd=64):
    q  = rmsnorm(x, gq) @ Wq.T    split into (B,H,T,hd)
    kn = rmsnorm(ctx, gkv);  k = kn @ Wk.T;  v = kn @ Wv.T
    s  = q k^T / sqrt(hd);  w = relu(s)^2 masked to |ctx_pos - idx[t]| <= 1
    o  = (w / max(sum_k w, 1e-6)) @ v
    out = x + o @ Wo.T

Sharding: 8 cores = (batch b = core//2) x (token half = core%2); T_loc=2048.
No collectives needed (cross-attention over full per-batch ctx).

Device-side layout: all projections consume *transposed* activations
(d on partitions) so scores come out with ctx position on partitions and
tokens on the free axis.  The per-token RMSNorm factor rs_x[t] cancels in
w/denom; the eps clamp is handled exactly via per-token row math
(g[t] = rs^2 / max(rs^2 * U, eps)) on (16, tch) tiles.  The per-ctx-row
factor rs_ctx folds into V (per-partition scale on psum copy) and into the
scores bias pass (per-partition scalar multiply).  A ones-column appended
to V makes the attention matmul emit the denominator U[t] as psum row 64.
Token chunks of 512 are pipelined end-to-end (q-proj -> attention ->
normalize -> o-proj + residual) to bound SBUF.
"""
import sys

sys.path.insert(0, "/opt/trn_rl_repo")

import numpy as np

B, T, KC, D, H, HD = 4, 4096, 256, 1024, 16, 64
N_CORES = 8
T_LOC = T * B // N_CORES  # 2048
TCH = 512
N_CHUNK = T_LOC // TCH    # 4
EPS_RMS = 1.1920929e-07
EPS_DEN = 1e-6
SCALE = 1.0 / 8.0         # 1/sqrt(hd)
NEG_BIG = 1.0e6

_CACHE = {}


def _build():
    import concourse.mybir as mybir
    import concourse.tile as tile
    from concourse import bacc
    from concourse.masks import make_identity

    f32 = mybir.dt.float32
    f32r = mybir.dt.float32r
    Alu = mybir.AluOpType
    Act = mybir.ActivationFunctionType

    nc = bacc.Bacc("TRN2", target_bir_lowering=False, debug=False)

    xT = nc.dram_tensor("xT", [D, T_LOC], f32, kind="ExternalInput")
    x_nat = nc.dram_tensor("x_nat", [T_LOC, D], f32, kind="ExternalInput")
    ctxT = nc.dram_tensor("ctxT", [D, KC], f32, kind="ExternalInput")
    ctx_nat = nc.dram_tensor("ctx_nat", [KC, D], f32, kind="ExternalInput")
    idx_row = nc.dram_tensor("idx_row", [1, T_LOC], f32, kind="ExternalInput")
    pcol = nc.dram_tensor("pcol", [128, 1], f32, kind="ExternalInput")
    wq_t = nc.dram_tensor("wq_t", [D, D], f32, kind="ExternalInput")
    wk_t = nc.dram_tensor("wk_t", [D, D], f32, kind="ExternalInput")
    wv_t = nc.dram_tensor("wv_t", [D, D], f32, kind="ExternalInput")
    wo_t = nc.dram_tensor("wo_t", [D, D], f32r, kind="ExternalInput")
    out_d = nc.dram_tensor("out", [T_LOC, D], f32, kind="ExternalOutput")

    with tile.TileContext(nc) as tc:
        with (
            tc.tile_pool(name="pw", bufs=1) as pw,
            tc.tile_pool(name="pbig", bufs=1) as pbig,
            tc.tile_pool(name="pxt", bufs=1) as pxt,
            tc.tile_pool(name="pstat", bufs=1) as pstat,
            tc.tile_pool(name="pscr", bufs=1) as pscr,
            tc.tile_pool(name="ptmp", bufs=3) as ptmp,
            tc.tile_pool(name="pnat", bufs=2) as pnat,
            tc.tile_pool(name="pdram", bufs=1, space="DRAM") as pdram,
            tc.tile_pool(name="ppq", bufs=2, space="PSUM") as ppq,
            tc.tile_pool(name="pps", bufs=4, space="PSUM") as pps,
            tc.tile_pool(name="ppa", bufs=2, space="PSUM") as ppa,
        ):
            # ---------------- phase A: setup ----------------
            wk_sb = pw.tile([128, 8 * D], f32, tag="wA")
            wv_sb = pw.tile([128, 8 * D], f32, tag="wB")
            for kd in range(8):
                nc.sync.dma_start(
                    wk_sb[:, D * kd : D * (kd + 1)], wk_t[128 * kd : 128 * (kd + 1), :]
                )
                nc.sync.dma_start(
                    wv_sb[:, D * kd : D * (kd + 1)], wv_t[128 * kd : 128 * (kd + 1), :]
                )
            ctxT_sb = pxt.tile([128, 8 * KC], f32, tag="xt")
            for kd in range(8):
                nc.sync.dma_start(
                    ctxT_sb[:, KC * kd : KC * (kd + 1)],
                    ctxT[128 * kd : 128 * (kd + 1), :],
                )

            eps_sb = pstat.tile([128, 1], f32, tag="eps")
            nc.vector.memset(eps_sb[:], EPS_RMS)
            pcol_sb = pstat.tile([128, 1], f32, tag="pcol")
            nc.sync.dma_start(pcol_sb[:], pcol[:])

            # rs_ctx per ctx row (2 tiles of 128); squares computed in place
            rs_ctx = []
            for kt in range(2):
                cn = pnat.tile([128, D], f32, tag="xnat")
                nc.sync.dma_start(cn[:], ctx_nat[128 * kt : 128 * (kt + 1), :])
                ssq = pstat.tile([128, 1], f32, tag=f"ssqc{kt}")
                nc.scalar.activation(cn[:], cn[:], Act.Square, accum_out=ssq[:])
                rt = pstat.tile([128, 1], f32, tag=f"rtc{kt}")
                nc.scalar.activation(
                    rt[:], ssq[:], Act.Sqrt, bias=eps_sb[:], scale=1.0 / D
                )
                rc = pstat.tile([128, 1], f32, tag=f"rsctx{kt}")
                nc.vector.reciprocal(rc[:], rt[:])
                rs_ctx.append(rc)

            # rs_x per token -> (1, T_LOC) row staged via PE transpose + DRAM
            ssq_stack = pstat.tile([128, 16], f32, tag="ssqstack")
            for j in range(16):
                xn = pnat.tile([128, D], f32, tag="xnat")
                nc.sync.dma_start(xn[:], x_nat[128 * j : 128 * (j + 1), :])
                nc.scalar.activation(
                    xn[:], xn[:], Act.Square, accum_out=ssq_stack[:, j : j + 1]
                )
            rt_stack = pstat.tile([128, 16], f32, tag="rtstack")
            nc.scalar.activation(
                rt_stack[:], ssq_stack[:], Act.Sqrt, bias=eps_sb[:], scale=1.0 / D
            )
            rs_stack = pstat.tile([128, 16], f32, tag="rsstack")
            nc.vector.reciprocal(rs_stack[:], rt_stack[:])
            ident = pstat.tile([128, 128], f32, tag="ident")
            make_identity(nc, ident[:])
            tr_ps = ppa.tile([16, 128], f32, tag="pa")
            nc.tensor.transpose(tr_ps[:], rs_stack[:], ident[:])
            tr_sb = pstat.tile([16, 128], f32, tag="trsb")
            nc.scalar.copy(tr_sb[:], tr_ps[:])
            rs_dram = pdram.tile([1, T_LOC], f32, tag="rsd")
            rs_wview = rs_dram[:].rearrange("a (p f) -> (a p) f", p=16, f=128)
            nc.sync.dma_start(rs_wview, tr_sb[:])
            rs_row = rs_dram[:]

            # kv projections
            v_sb = pbig.tile([128, 2 * H * (HD + 1)], f32r, tag="v")  # slots of 65
            for kt in range(2):
                for nch in range(2):
                    ps_v = ppq.tile([128, 512], f32, tag="pj")
                    for kd in range(8):
                        nc.tensor.matmul(
                            ps_v[:],
                            ctxT_sb[:, KC * kd + 128 * kt : KC * kd + 128 * (kt + 1)],
                            wv_sb[:, D * kd + 512 * nch : D * kd + 512 * (nch + 1)],
                            start=(kd == 0), stop=(kd == 7),
                        )
                    for hh in range(8):  # heads 8*nch .. 8*nch+7
                        h = 8 * nch + hh
                        slot = (HD + 1) * (H * kt + h)
                        nc.scalar.activation(
                            v_sb[:, slot : slot + HD],
                            ps_v[:, 64 * hh : 64 * (hh + 1)],
                            Act.Copy, scale=rs_ctx[kt][:],
                        )
                        nc.vector.memset(v_sb[:, slot + HD : slot + HD + 1].bitcast(f32), 1.0)

            k_sb = pbig.tile([128, 8 * KC], f32, tag="k")
            for mr in range(8):
                ps_k = ppq.tile([128, 512], f32, tag="pj")
                for kd in range(8):
                    nc.tensor.matmul(
                        ps_k[:, 0:KC],
                        wk_sb[:, D * kd + 128 * mr : D * kd + 128 * (mr + 1)],
                        ctxT_sb[:, KC * kd : KC * (kd + 1)],
                        start=(kd == 0), stop=(kd == 7),
                    )
                nc.scalar.activation(
                    k_sb[:, KC * mr : KC * (mr + 1)], ps_k[:, 0:KC],
                    Act.Copy, scale=SCALE,
                )

            # ---------------- phase B: per 512-token chunk, end to end ----------
            wq_sb = pw.tile([128, 8 * D], f32, tag="wB")  # reuses wv slot
            for kd in range(8):
                nc.sync.dma_start(
                    wq_sb[:, D * kd : D * (kd + 1)], wq_t[128 * kd : 128 * (kd + 1), :]
                )
            wo_sb = pw.tile([128, 8 * D], f32r, tag="wA")  # reuses wk slot
            for kd in range(8):
                nc.sync.dma_start(
                    wo_sb[:, D * kd : D * (kd + 1)], wo_t[128 * kd : 128 * (kd + 1), :]
                )

            for c in range(N_CHUNK):
                tsl = slice(TCH * c, TCH * (c + 1))
                # band-mask bias for this chunk, kctx-layout
                idx_b = pscr.tile([128, TCH], f32, tag="idxb")
                nc.gpsimd.dma_start(
                    idx_b[:], idx_row[0:1, tsl].to_broadcast((128, TCH))
                )
                bias_kt = []
                for kt in range(2):
                    dtile = pscr.tile([128, TCH], f32, tag="dtile")
                    nc.vector.tensor_scalar(
                        dtile[:], idx_b[:], pcol_sb[:], float(128 * kt),
                        op0=Alu.subtract, op1=Alu.subtract,
                    )
                    atile = pscr.tile([128, TCH], f32, tag="atile")
                    nc.scalar.activation(atile[:], dtile[:], Act.Abs)
                    btile = pscr.tile([128, TCH], f32, tag=f"bias{kt}")
                    nc.vector.tensor_scalar(
                        btile[:], atile[:], 1.5, 1.0, op0=Alu.is_le, op1=Alu.subtract
                    )
                    nc.vector.tensor_scalar_mul(btile[:], btile[:], NEG_BIG)
                    bias_kt.append(btile)

                # rs_x^2 rows for this chunk (all 16 partitions identical)
                sq16 = pscr.tile([16, TCH], f32, tag="sq16")
                nc.gpsimd.dma_start(
                    sq16[:], rs_row[0:1, tsl].to_broadcast((16, TCH))
                )
                nc.vector.tensor_tensor(sq16[:], sq16[:], sq16[:], Alu.mult)

                # q projection (transposed): qT_c[128 = 2 heads, t]
                xt_sb = pxt.tile([128, 8 * TCH], f32, tag="xt")
                for kd in range(8):
                    nc.sync.dma_start(
                        xt_sb[:, TCH * kd : TCH * (kd + 1)],
                        xT[128 * kd : 128 * (kd + 1), tsl],
                    )
                qT_c = pbig.tile([128, 8 * TCH], f32, tag="qT")
                for mr in range(8):
                    ps_q = ppq.tile([128, 512], f32, tag="pj")
                    for kd in range(8):
                        nc.tensor.matmul(
                            ps_q[:],
                            wq_sb[:, D * kd + 128 * mr : D * kd + 128 * (mr + 1)],
                            xt_sb[:, TCH * kd : TCH * (kd + 1)],
                            start=(kd == 0), stop=(kd == 7),
                        )
                    nc.scalar.copy(qT_c[:, TCH * mr : TCH * (mr + 1)], ps_q[:])

                # attention per head
                attnT_c = pbig.tile([128, 8 * TCH], f32r, tag="attnT")
                U16 = pstat.tile([16, TCH], f32, tag="U16")
                for h in range(H):
                    mr, half = h // 2, h % 2
                    hsl = slice(64 * half, 64 * (half + 1))
                    u_tiles = []
                    for kt in range(2):
                        ps_s = pps.tile([128, TCH], f32, tag="ps")
                        nc.tensor.matmul(
                            ps_s[:],
                            k_sb[hsl, KC * mr + 128 * kt : KC * mr + 128 * (kt + 1)],
                            qT_c[hsl, TCH * mr : TCH * (mr + 1)],
                            start=True, stop=True,
                        )
                        s_m = ptmp.tile([128, TCH], f32, tag="sm")
                        nc.vector.scalar_tensor_tensor(
                            s_m[:], ps_s[:], rs_ctx[kt][:], bias_kt[kt][:],
                            op0=Alu.mult, op1=Alu.add,
                        )
                        u = ptmp.tile([128, TCH], f32r, tag="u")
                        nc.vector.scalar_tensor_tensor(
                            u[:], s_m[:], 0.0, s_m[:], op0=Alu.max, op1=Alu.mult
                        )
                        u_tiles.append(u)
                    ps_a = ppa.tile([HD + 1, TCH], f32, tag="pa")
                    for kt in range(2):
                        slot = (HD + 1) * (H * kt + h)
                        nc.tensor.matmul(
                            ps_a[:],
                            v_sb[:, slot : slot + HD + 1],
                            u_tiles[kt][:],
                            start=(kt == 0), stop=(kt == 1),
                        )
                    nc.scalar.copy(
                        attnT_c[hsl, TCH * mr : TCH * (mr + 1)], ps_a[0:HD, :]
                    )
                    stu = ptmp.tile([HD + 1, TCH], f32, tag="stu")
                    nc.scalar.copy(stu[HD : HD + 1, :], ps_a[HD : HD + 1, :])
                    nc.sync.dma_start(U16[h : h + 1, :], stu[HD : HD + 1, :])

                # exact per-token normalizer g = rs^2 / max(rs^2 * U, eps)
                g16 = pstat.tile([16, TCH], f32, tag="g16")
                nc.vector.tensor_tensor(g16[:], sq16[:], U16[:], Alu.mult)
                nc.vector.tensor_scalar_max(g16[:], g16[:], EPS_DEN)
                nc.vector.reciprocal(g16[:], g16[:])
                nc.vector.tensor_tensor(g16[:], g16[:], sq16[:], Alu.mult)
                g_dram = pdram.tile([16, TCH], f32, tag="gd")
                nc.sync.dma_start(g_dram[:], g16[:])
                for h in range(H):
                    mr, half = h // 2, h % 2
                    hsl = slice(64 * half, 64 * (half + 1))
                    gb = ptmp.tile([128, TCH], f32, tag="gb")
                    nc.gpsimd.dma_start(
                        gb[:], g_dram[h : h + 1, :].to_broadcast((128, TCH))
                    )
                    sl = attnT_c[hsl, TCH * mr : TCH * (mr + 1)]
                    nc.vector.tensor_tensor(sl, sl, gb[hsl, :], Alu.mult)

                # o-projection + residual for this chunk (4 token tiles)
                for mm in range(4):
                    m = 4 * c + mm
                    xn = pnat.tile([128, D], f32, tag="xnat")
                    nc.sync.dma_start(xn[:], x_nat[128 * m : 128 * (m + 1), :])
                    o_sb = pnat.tile([128, D], f32, tag="osb")
                    for nch in range(2):
                        ps_o = ppq.tile([128, 512], f32, tag="pj")
                        for kd in range(8):
                            nc.tensor.matmul(
                                ps_o[:],
                                attnT_c[:, TCH * kd + 128 * mm : TCH * kd + 128 * (mm + 1)],
                                wo_sb[:, D * kd + 512 * nch : D * kd + 512 * (nch + 1)],
                                start=(kd == 0), stop=(kd == 7),
                            )
                        nc.vector.tensor_tensor(
                            o_sb[:, 512 * nch : 512 * (nch + 1)],
                            ps_o[:],
                            xn[:, 512 * nch : 512 * (nch + 1)],
                            Alu.add,
                        )
                    nc.sync.dma_start(out_d[128 * m : 128 * (m + 1), :], o_sb[:])

    nc.compile()
    return nc


def kernel(**inputs):
    from concourse.bass_utils import run_bass_kernel_spmd

    if "nc" not in _CACHE:
        _CACHE["nc"] = _build()
    nc = _CACHE["nc"]

    x = np.asarray(inputs["x_tokens"], dtype=np.float32)
    ctx = np.asarray(inputs["ctx"], dtype=np.float32)
    idx = np.asarray(inputs["ctx_index_per_token"])
    gq = np.asarray(inputs["gamma_q"], dtype=np.float32)
    gkv = np.asarray(inputs["gamma_kv"], dtype=np.float32)
    # fold rmsnorm gammas into the projection weights (diagonal absorb)
    Wq = np.asarray(inputs["Wq"], dtype=np.float32) * gq[None, :]
    Wk = np.asarray(inputs["Wk"], dtype=np.float32) * gkv[None, :]
    Wv = np.asarray(inputs["Wv"], dtype=np.float32) * gkv[None, :]
    Wo = np.asarray(inputs["Wo"], dtype=np.float32)

    wq_T = np.ascontiguousarray(Wq.T)
    wk_T = np.ascontiguousarray(Wk.T)
    wv_T = np.ascontiguousarray(Wv.T)
    wo_T = np.ascontiguousarray(Wo.T)
    pcol = np.arange(128, dtype=np.float32)[:, None].copy()

    in_maps = []
    for c in range(N_CORES):
        b, th = c // 2, c % 2
        sl = slice(th * T_LOC, (th + 1) * T_LOC)
        x_nat = np.ascontiguousarray(x[b, sl])
        in_maps.append({
            "xT": np.ascontiguousarray(x_nat.T),
            "x_nat": x_nat,
            "ctxT": np.ascontiguousarray(ctx[b].T),
            "ctx_nat": np.ascontiguousarray(ctx[b]),
            "idx_row": np.ascontiguousarray(
                idx[b, sl].astype(np.float32)[None, :]
            ),
            "pcol": pcol,
            "wq_t": wq_T, "wk_t": wk_T, "wv_t": wv_T, "wo_t": wo_T,
        })

    _CACHE["in_maps"] = in_maps
    res = run_bass_kernel_spmd(nc, in_maps, list(range(N_CORES)))
    out = np.empty((B, T, D), dtype=np.float32)
    for c in range(N_CORES):
        b, th = c // 2, c % 2
        out[b, th * T_LOC : (th + 1) * T_LOC] = res.results[c]["out"]
    return out


def run_traced(inputs):
    """Run once with NTFF tracing; returns max per-core exec time in ns."""
    kernel(**inputs)  # ensure compiled
    nc = _CACHE["nc"]
    from concourse.bass_utils import run_bass_kernel_spmd
    import tempfile
    in_maps = _CACHE.get("in_maps")
    if in_maps is None:
        return None
    res = run_bass_kernel_spmd(
        nc, in_maps, list(range(N_CORES)), trace=True,
        tmpdir=tempfile.mkdtemp(prefix="bass_trace_"),
    )
    return res.exec_time_ns
